# revision 4
# baseline (speedup 1.0000x reference)
"""Trainium2 Bass kernel for nn_ChebyshevLayer (gnn_message_passing).

Strategy (8 NeuronCores, SPMD):
- X0 = transpose(x,(1,2,0)).reshape(M, FIN*N) -> [50000, 128]; pad rows to 50176.
- Rows are dealt to cores by (qA,qB) "class cell" so every core has an identical
  reduce-segment structure (one shared NEFF). qA/qB = ceil(row nnz count in each
  column-half / 4); the column space is split in half so gather indices fit int16.
- Each SpMM launch: per-core transposed HBM dma_gather of X[col] rows (bf16,
  256B elements), partition_broadcast of vals, DVE multiply, strided DVE
  segment reduces into Y^T [128=(f,n), rows], then X_next = alpha*Y - X_prev.
- Host concatenates per-core row slices between the 3 SpMM launches (pure data
  movement) and re-feeds the full X as the next gather source.
- Final launch: einsum as 4 accumulated PE matmuls with block-diagonal W-hat
  plus a K=1 ones-matmul adding the bias.
"""

import numpy as np
import ml_dtypes

import concourse.bacc as bacc
import concourse.mybir as mybir
from concourse.tile import TileContext
from concourse.bass_utils import run_bass_kernel_spmd
from concourse.library_config import mlp

N, M, FIN, FOUT, KCH = 8, 50000, 16, 32, 4
MP = 50176               # padded row count (392*128)
HALF = MP // 2           # 25088 (int16 gather index limit per half)
PW = 4                   # class pad width
CHUNK = 4096             # gather slots per chunk
NCORE = 8
BF16 = ml_dtypes.bfloat16


def _ceil(a, b):
    return -(-a // b)


class Plan:
    pass


def build_plan(rows, cols):
    """Shared (all-core) chunk/segment plan + per-core row assignment."""
    p = Plan()
    cA = np.bincount(rows[cols < HALF], minlength=MP).astype(np.int64)
    cB = np.bincount(rows[cols >= HALF], minlength=MP).astype(np.int64)
    qA = _ceil(cA, PW)
    qB = _ceil(cB, PW)
    cell = qA * 64 + qB
    order = np.argsort(cell, kind="stable")
    cell_s = cell[order]
    ucells, starts_u, Ncell = np.unique(cell_s, return_index=True, return_counts=True)
    Pcell = _ceil(Ncell, NCORE)
    Rp = int(Pcell.sum())
    R128 = _ceil(Rp, 128) * 128
    cell_base = np.concatenate([[0], np.cumsum(Pcell)[:-1]])

    dev2glob = np.full((NCORE, Rp), -1, np.int64)
    j_of = np.repeat(np.arange(len(ucells)), Ncell)
    i_of = np.arange(len(order)) - np.repeat(starts_u, Ncell)
    dev2glob[i_of % NCORE, cell_base[j_of] + i_of // NCORE] = order

    qA_c = (ucells // 64).astype(np.int64)
    qB_c = (ucells % 64).astype(np.int64)

    # chunks: list of dicts {half, segs:[(soff, n, w, yoff, write)]}
    chunks = []
    base = [np.zeros(Rp, np.int64), np.zeros(Rp, np.int64)]  # per-half slot base per dev row
    wdev = [np.repeat(qA_c * PW, Pcell), np.repeat(qB_c * PW, Pcell)]
    for half in (0, 1):
        cur = CHUNK  # force new chunk at start of each half
        ch = None
        for j in range(len(ucells)):
            w = int((qA_c if half == 0 else qB_c)[j]) * PW
            if w == 0:
                continue
            wr = True if half == 0 else (qA_c[j] == 0)
            r = int(cell_base[j])
            rem = int(Pcell[j])
            while rem > 0:
                nfit = (CHUNK - cur) // w
                if nfit == 0:
                    ch = {"half": half, "segs": []}
                    chunks.append(ch)
                    cur = 0
                    nfit = CHUNK // w
                n = min(rem, nfit)
                ch["segs"].append((cur, n, w, r, wr))
                gslot = (len(chunks) - 1) * CHUNK + cur
                base[half][r:r + n] = gslot + np.arange(n) * w
                cur += n * w
                r += n
                rem -= n
    p.chunks = chunks
    p.nch = len(chunks)
    p.stot = p.nch * CHUNK
    p.Rp, p.R128 = Rp, R128
    p.dev2glob = dev2glob
    p.baseA, p.baseB = base[0], base[1]
    return p


def build_core_inputs(p, k, rows, cols, vals):
    """Per-core idx/vals streams following the shared plan."""
    g2d = np.full(MP, -1, np.int64)
    mine = p.dev2glob[k] >= 0
    g2d[p.dev2glob[k][mine]] = np.nonzero(mine)[0]
    idx_arr = np.zeros(p.stot, np.int16)
    val_arr = np.zeros(p.stot, np.float32)
    for half in (0, 1):
        m = ((cols < HALF) if half == 0 else (cols >= HALF)) & (g2d[rows] >= 0)
        r, c, v = rows[m], cols[m], vals[m]
        o = np.argsort(r, kind="stable")
        r, c, v = r[o], c[o], v[o]
        first = np.searchsorted(r, r, side="left")
        rank = np.arange(len(r)) - first
        basearr = p.baseA if half == 0 else p.baseB
        pos = basearr[g2d[r]] + rank
        idx_arr[pos] = (c if half == 0 else c - HALF).astype(np.int16)
        val_arr[pos] = v
    idx_w = np.tile(
        np.hstack([idx_arr[i * CHUNK:(i + 1) * CHUNK].reshape(-1, 16).T
                   for i in range(p.nch)]), (8, 1)).astype(np.int16)
    vals_t = val_arr.reshape(1, p.stot).astype(BF16)
    return idx_w, vals_t


def build_spmm_nc(p, alpha=2.0, reps=1):
    nc = bacc.Bacc("TRN2")
    dt = mybir.dt
    xsrc = nc.dram_tensor("xsrc", [MP, 128], dt.bfloat16, kind="ExternalInput")
    idx = nc.dram_tensor("idx", [128, p.stot // 16], dt.int16, kind="ExternalInput")
    valst = nc.dram_tensor("vals", [1, p.stot], dt.bfloat16, kind="ExternalInput")
    xprev = nc.dram_tensor("xprev", [128, p.R128], dt.float32, kind="ExternalInput")
    xnext = nc.dram_tensor("xnext", [128, p.R128], dt.float32, kind="ExternalOutput")
    with TileContext(nc) as tc:
        nc.gpsimd.load_library(mlp)
        with tc.tile_pool(name="io", bufs=1) as iop, \
             tc.tile_pool(name="g", bufs=2) as gp, \
             tc.tile_pool(name="vs", bufs=2) as vsp, \
             tc.tile_pool(name="vb", bufs=2) as vbp, \
             tc.tile_pool(name="tmp", bufs=2) as tp, \
             tc.tile_pool(name="y", bufs=1) as yp:
            idx_sb = iop.tile([128, p.stot // 16], dt.int16)
            nc.sync.dma_start(idx_sb[:], idx[:])
            xp_sb = iop.tile([128, p.R128], dt.float32)
            nc.sync.dma_start(xp_sb[:], xprev[:])
            for _ in range(reps):
                Y = yp.tile([128, p.R128], dt.float32, tag="Y")
                nc.vector.memset(Y[:], 0.0)
                for ci, ch in enumerate(p.chunks):
                    g = gp.tile([128, 1, CHUNK], dt.bfloat16, tag="g")
                    src = xsrc[0:HALF, :] if ch["half"] == 0 else xsrc[HALF:MP, :]
                    nc.gpsimd.dma_gather(
                        g[:], src, idx_sb[:, ci * CHUNK // 16:(ci + 1) * CHUNK // 16],
                        CHUNK, CHUNK, 128, transpose=True, single_packet=False)
                    vs = vsp.tile([1, CHUNK], dt.bfloat16, tag="vs")
                    nc.sync.dma_start(vs[:], valst[:, ci * CHUNK:(ci + 1) * CHUNK])
                    vb = vbp.tile([128, CHUNK], dt.bfloat16, tag="vb")
                    nc.gpsimd.partition_broadcast(vb[:], vs[:])
                    g2 = g[:].rearrange("p a c -> p (a c)")
                    nc.vector.tensor_tensor(g2, g2, vb[:], mybir.AluOpType.mult)
                    for (soff, n, w, yoff, wr) in ch["segs"]:
                        gr = g[:, :, soff:soff + n * w].rearrange(
                            "p a (n w) -> p (a n) w", w=w)
                        if wr:
                            nc.vector.tensor_reduce(
                                Y[:, yoff:yoff + n], gr,
                                mybir.AxisListType.X, mybir.AluOpType.add)
                        else:
                            t = tp.tile([128, n], dt.float32, tag="t")
                            nc.vector.tensor_reduce(
                                t[:], gr, mybir.AxisListType.X, mybir.AluOpType.add)
                            nc.vector.tensor_tensor(
                                Y[:, yoff:yoff + n], Y[:, yoff:yoff + n], t[:],
                                mybir.AluOpType.add)
                nc.vector.tensor_scalar(Y[:], Y[:], float(alpha), None,
                                        mybir.AluOpType.mult)
                xn = yp.tile([128, p.R128], dt.float32, tag="xn")
                nc.vector.tensor_tensor(xn[:], Y[:], xp_sb[:],
                                        mybir.AluOpType.subtract)
                nc.sync.dma_start(xnext[:, :], xn[:])
    nc.compile()
    return nc


def build_einsum_nc(R128, reps=1):
    from contextlib import ExitStack
    nc = bacc.Bacc("TRN2")
    dt = mybir.dt
    ts = [nc.dram_tensor(f"t{i}", [128, R128], dt.bfloat16, kind="ExternalInput")
          for i in range(KCH)]
    wm = nc.dram_tensor("wm", [KCH * 128, 256], dt.bfloat16, kind="ExternalInput")
    onesb = nc.dram_tensor("onesb", [1, 128], dt.bfloat16, kind="ExternalInput")
    bvec = nc.dram_tensor("bvec", [1, 256], dt.bfloat16, kind="ExternalInput")
    outt = nc.dram_tensor("outt", [R128, 256], dt.float32, kind="ExternalOutput")
    ntile = R128 // 128
    with TileContext(nc) as tc:
        with tc.tile_pool(name="io", bufs=1) as iop, \
             tc.tile_pool(name="ps", bufs=4, space="PSUM") as psp, \
             tc.tile_pool(name="o", bufs=4) as osp:
            t_sb = []
            for i in range(KCH):
                tt = iop.tile([128, R128], dt.bfloat16, tag=f"t{i}")
                nc.sync.dma_start(tt[:], ts[i][:])
                t_sb.append(tt)
            w_sb = []
            for i in range(KCH):
                wt = iop.tile([128, 256], dt.bfloat16, tag=f"w{i}")
                nc.sync.dma_start(wt[:], wm[i * 128:(i + 1) * 128, :])
                w_sb.append(wt)
            on_sb = iop.tile([1, 128], dt.bfloat16)
            nc.sync.dma_start(on_sb[:], onesb[:])
            bv_sb = iop.tile([1, 256], dt.bfloat16)
            nc.sync.dma_start(bv_sb[:], bvec[:])
            for _ in range(reps):
                for t in range(ntile):
                    ps = psp.tile([128, 256], dt.float32, tag="ps")
                    for k in range(KCH):
                        nc.tensor.matmul(ps[:],
                                         t_sb[k][:, t * 128:(t + 1) * 128],
                                         w_sb[k][:], start=(k == 0), stop=False)
                    nc.tensor.matmul(ps[:], on_sb[:], bv_sb[:],
                                     start=False, stop=True)
                    o = osp.tile([128, 256], dt.float32, tag="o")
                    nc.vector.tensor_copy(o[:], ps[:])
                    nc.sync.dma_start(outt[t * 128:(t + 1) * 128, :], o[:])
    nc.compile()
    return nc


_CACHE = {}


def _run(nc, in_maps):
    return run_bass_kernel_spmd(nc, in_maps, core_ids=list(range(NCORE)))


def kernel(x, l_vals, w, b, l_row, l_col, _timing=None):
    x = np.asarray(x, np.float32)
    l_vals = np.asarray(l_vals, np.float32)
    w = np.asarray(w, np.float32)
    b = np.asarray(b, np.float32)
    rows = np.asarray(l_row).astype(np.int64)
    cols = np.asarray(l_col).astype(np.int64)

    p = build_plan(rows, cols)
    key = (p.nch, p.R128)
    if key not in _CACHE:
        _CACHE[key] = (build_spmm_nc(p, 1.0), build_spmm_nc(p, 2.0), build_einsum_nc(p.R128))
    nc_spmm1, nc_spmm2, nc_ein = _CACHE[key]

    # X0 full [MP, 128] f32 (rows padded with zeros)
    X0 = np.zeros((MP, 128), np.float32)
    X0[:M] = x.transpose(1, 2, 0).reshape(M, FIN * N)

    core_in = [build_core_inputs(p, k, rows, cols, l_vals) for k in range(NCORE)]

    # per-core X^T slices in device order (virtual rows -> 0)
    def dev_slices(Xfull):
        out = []
        for k in range(NCORE):
            s = np.zeros((128, p.R128), np.float32)
            mine = p.dev2glob[k] >= 0
            s[:, :p.Rp][:, mine] = Xfull[p.dev2glob[k][mine]].T
            out.append(s)
        return out

    def assemble(slices):
        Xf = np.zeros((MP, 128), np.float32)
        for k in range(NCORE):
            mine = p.dev2glob[k] >= 0
            Xf[p.dev2glob[k][mine]] = slices[k][:, :p.Rp][:, mine].T
        return Xf

    Xt_slices = [dev_slices(X0)]          # T0 slices
    Xcur = X0
    zeros_sl = [np.zeros((128, p.R128), np.float32)] * NCORE

    times = []
    import time
    for it in range(KCH - 1):
        ncs = nc_spmm1 if it == 0 else nc_spmm2
        xprev_sl = zeros_sl if it == 0 else Xt_slices[it - 1]
        in_maps = [{
            "xsrc": Xcur.astype(BF16),
            "idx": core_in[k][0],
            "vals": core_in[k][1],
            "xprev": xprev_sl[k],
        } for k in range(NCORE)]
        t0 = time.time()
        res = _run(ncs, in_maps)
        times.append(time.time() - t0)
        new_sl = [res.results[k]["xnext"] for k in range(NCORE)]
        Xt_slices.append(new_sl)
        Xcur = assemble(new_sl)

    # einsum
    wmat = np.zeros((KCH * 128, 256), np.float32)
    for k in range(KCH):
        for pp in range(128):
            f, n = pp // 8, pp % 8
            wmat[k * 128 + pp, n * 32:(n + 1) * 32] = w[f, k, :]
    bv = np.tile(b.reshape(1, FOUT), (1, 8)).astype(np.float32)
    ein_maps = [{
        **{f"t{i}": Xt_slices[i][k].astype(BF16) for i in range(KCH)},
        "wm": wmat.astype(BF16),
        "onesb": np.ones((1, 128), BF16),
        "bvec": bv.astype(BF16),
    } for k in range(NCORE)]
    t0 = time.time()
    res = _run(nc_ein, ein_maps)
    times.append(time.time() - t0)

    out = np.zeros((N, M, FOUT), np.float32)
    for k in range(NCORE):
        o = res.results[k]["outt"]            # [R128, 256]
        mine = p.dev2glob[k] >= 0
        rows_k = p.dev2glob[k][mine]
        real = rows_k < M
        o3 = o[:p.Rp][mine][real].reshape(-1, N, FOUT)   # [nrows, n, o]
        out[:, rows_k[real], :] = o3.transpose(1, 0, 2)
    if _timing is not None:
        _timing.extend(times)
    return out
